# revision 24
# baseline (speedup 1.0000x reference)
# Multi-head causal attention for 8 Trainium2 NeuronCores (Bass/Tile).
#
# Problem: q,k,v [2,16,2048,64] f32, bool mask [1,1,2048,2048] (True = masked,
# additive -1e4 bias before softmax in the reference).
#
# Sharding: batch*heads = 32 items, 4 per core (pure data/head parallel, no
# communication).
#
# Per-core kernel (per head), all in "transposed score" layout so softmax'd
# probabilities come out of the ScalarEngine already laid out for the PV
# matmul (keys on partitions):
#   - Q,K arrive pre-transposed from the host ([head, d, seq] fp16), V arrives
#     pre-arranged in its exact SBUF layout [128, nb, 65] with a ones column
#     baked in (so every input DMA is one large contiguous transfer -- the
#     strided V gather used to serialize ~13us of DMA at kernel start).
#   - Per key-block j: S^T_j = K_j Q^T via matmul into PSUM [128, <=1024];
#     exp on ScalarE with the 1/sqrt(64) scale folded in (no row-max
#     subtraction: |scores| <= ~7, exp is safe in f32, and softmax is
#     shift-invariant so the result matches the reference).
#   - Mask handling, decided on the host per 128x128 block from the actual
#     mask input: fully-masked blocks are skipped outright (their probs
#     underflow to exactly 0 in the reference too); mixed blocks multiply
#     the probabilities by a 0/1 keep-tile (equivalent to the -1e4 bias:
#     exp(s - 1e4) == 0 exactly in f32) on the otherwise idle GpSimd engine.
#   - PV accumulates O^T [65, q] in PSUM over key-blocks, with V augmented
#     by a ones-column so row 64 of the accumulator is the softmax
#     denominator for free.
#   - Epilogue (all fp16 staging -- keeps the PE transposes at 1 cyc/col):
#     GpSimd copies the PSUM accumulator halves to SBUF, TensorE transposes
#     O^T back, DVE multiplies by the reciprocal denominator (gathered to
#     [128,16] via a tiny SBUF->SBUF DMA), DMA out.
#   - The PE instruction stream is chained (sync=False deps) in a software-
#     pipelined order so the TensorEngine -- the bottleneck at its throttled
#     sustained rate of ~1 col / 0.83ns -- never waits: QK_j+1 before PV_j,
#     epilogue transposes of head h slotted into head h+1's QK stream.
import numpy as np
from contextlib import ExitStack

B, H, S, D = 2, 16, 2048, 64
NCORES = 8
BH = B * H
HPC = BH // NCORES  # heads per core
BLK = 128
NB = S // BLK  # 16
VW = D + 1  # V columns + ones column
SCALE = 1.0 / 8.0  # 1/sqrt(D)

FREE, SKIP, BIAS = 0, 1, 2

_cache = {}


def _plan_from_mask(mask):
    """Classify 128x128 mask blocks; build unique 0/1 keep-tiles ([key, query]
    orientation) for the mixed blocks."""
    mask2d = np.asarray(mask).reshape(S, S).astype(bool)
    m = mask2d.reshape(NB, BLK, NB, BLK)
    anyb = m.any(axis=(1, 3))
    allb = m.all(axis=(1, 3))
    codes = np.where(allb, SKIP, np.where(anyb, BIAS, FREE)).astype(np.int64)
    # A query row whose whole key range is masked sees a constant bias, which
    # softmax ignores -- the reference then equals unmasked attention. Treat
    # whole such q-blocks as unmasked.
    fq = mask2d.all(axis=1).reshape(NB, BLK).all(axis=1)
    codes[fq, :] = FREE
    tiles = {}
    tile_idx = np.full((NB, NB), -1, dtype=np.int64)
    for qb in range(NB):
        for kb in range(NB):
            if codes[qb, kb] != BIAS:
                continue
            t = np.ascontiguousarray(
                (~mask2d[qb * BLK:(qb + 1) * BLK, kb * BLK:(kb + 1) * BLK].T)
            ).astype(np.float32)
            key = t.tobytes()
            if key not in tiles:
                tiles[key] = (len(tiles), t)
            tile_idx[qb, kb] = tiles[key][0]
    if tiles:
        bt = np.stack([t for _, t in sorted(tiles.values())], axis=0)
    else:
        bt = np.zeros((1, BLK, BLK), np.float32)
    return codes, tile_idx, bt


def _ceil_pieces(c0, c1, step):
    out = []
    c = c0
    while c < c1:
        out.append((c, min(c + step, c1)))
        c = out[-1][1]
    return out


def _runs(blocks):
    """Contiguous runs from a sorted list of block indices."""
    runs = []
    for i in blocks:
        if runs and runs[-1][1] == i:
            runs[-1][1] = i + 1
        else:
            runs.append([i, i + 1])
    return [tuple(r) for r in runs]


def build_nc(codes, tile_idx, n_bt, mmdt_name="float16"):
    import concourse.bass as bass
    import concourse.mybir as mybir
    import concourse.tile as tile
    from concourse import bacc
    from concourse.masks import make_identity
    from concourse.tile_rust import add_dep_helper

    dt = mybir.dt
    f32 = dt.float32
    mmdt = getattr(dt, mmdt_name)
    Exp = mybir.ActivationFunctionType.Exp
    mult = mybir.AluOpType.mult

    # Per key-block: which q-blocks participate.
    active = {j: [i for i in range(NB) if codes[i, j] != SKIP] for j in range(NB)}
    for i in range(NB):
        assert any(codes[i, j] != SKIP for j in range(NB)), (
            "query block with all key blocks masked should be impossible"
        )
    # PV PSUM accumulation start/stop must be managed per 512-column PSUM
    # bank (4 q-blocks): first/last key-block writing each bank.
    NBANK = 4
    bank_first = {}
    bank_last = {}
    for bank in range(NBANK):
        js = [
            j
            for j in range(NB)
            if any(codes[i, j] != SKIP for i in range(bank * 4, bank * 4 + 4))
        ]
        bank_first[bank] = js[0]
        bank_last[bank] = js[-1]

    nc = bacc.Bacc("TRN2", target_bir_lowering=False, debug=False, num_devices=NCORES)
    # Q and K arrive pre-transposed ([head, d, seq]) and V pre-arranged in
    # its SBUF layout ([128, nb*65] with the ones column baked in), all
    # pre-cast to the matmul dtype -- every load is one contiguous DMA.
    qt_d = nc.dram_tensor("qt", [HPC, D, S], mmdt, kind="ExternalInput").ap()
    kt_d = nc.dram_tensor("kt", [HPC, D, S], mmdt, kind="ExternalInput").ap()
    v_d = nc.dram_tensor("v", [HPC, BLK, NB, VW], mmdt, kind="ExternalInput").ap()
    bt_d = nc.dram_tensor("bt", [n_bt, BLK, BLK], mmdt, kind="ExternalInput").ap()
    o_d = nc.dram_tensor("o", [HPC, S, D], mmdt, kind="ExternalOutput").ap()

    with tile.TileContext(nc) as tc, ExitStack() as ctx:
        const = ctx.enter_context(tc.tile_pool(name="const", bufs=1))
        ldpool = ctx.enter_context(tc.tile_pool(name="ld", bufs=HPC))
        tpool = ctx.enter_context(tc.tile_pool(name="tp", bufs=HPC))
        ppool = ctx.enter_context(tc.tile_pool(name="pp", bufs=3))
        otpool = ctx.enter_context(tc.tile_pool(name="ot", bufs=2))
        smpool = ctx.enter_context(tc.tile_pool(name="sm", bufs=3))
        outpool = ctx.enter_context(tc.tile_pool(name="ob", bufs=4))
        # PSUM budget (8 banks x 2KB/partition): sc 3 banks + pv 4 + rt 1
        # (the denominator transpose shares the rt bank at cols 512:520).
        # rt/dnt in their own rings (not the pv ring) so the next head's PV
        # accumulation never serializes behind this head's epilogue drain.
        scpool = ctx.enter_context(tc.tile_pool(name="sc", bufs=3, space="PSUM"))
        pvpool = ctx.enter_context(tc.tile_pool(name="pv", bufs=4, space="PSUM"))
        rtpool = ctx.enter_context(tc.tile_pool(name="rt", bufs=1, space="PSUM"))

        identm = const.tile([BLK, BLK], mmdt, tag="identm")
        make_identity(nc, identm[:])

        # PE-order bookkeeping: weight reloads cost ~330ns per stationary
        # swap, so same-weight matmuls must run adjacently. We collect the
        # PE instructions and chain them (sync=False deps) in a software-
        # pipelined order: QK_j+1 before PV_j, epilogue transposes of head h
        # slotted early into head h+1's stream.
        # ---- input loads for all heads, hoisted ahead of the compute so no
        # load DMA trigger ever sits behind a blocking compute instruction on
        # its engine queue. The DMA hardware round-robins active transfers,
        # so a transfer's latency scales with how many are in flight: head
        # 0's loads are split fine (k block 0 and the first q piece finish
        # fast, letting the PE start ~4us earlier) and later heads' loads are
        # gated on head 0's first matmul (via add_dep below) so they don't
        # steal preamble bandwidth.
        qts, kts, vnos, load_insts = [], [], [], []
        for h in range(HPC):
            lds = []
            qt = tpool.tile([D, S], mmdt, tag="qt", name=f"qt{h}")
            kt = tpool.tile([D, S], mmdt, tag="kt", name=f"kt{h}")
            if h == 0:
                # head 0 split fine: k block 0 and small q pieces finish
                # fast so the first QK matmul fires ASAP; kt on the scalar
                # queue (idle in the preamble), qt on sync
                lds.append(nc.scalar.dma_start(out=kt[:, 0:BLK], in_=kt_d[h][:, 0:BLK]))
                for (c0, c1) in [(0, 256), (256, 512), (512, 1024), (1024, 1536), (1536, 2048)]:
                    lds.append(nc.sync.dma_start(
                        out=qt[:, c0:c1], in_=qt_d[h][:, c0:c1]
                    ))
                lds.append(nc.scalar.dma_start(out=kt[:, BLK:], in_=kt_d[h][:, BLK:]))
            else:
                # later heads entirely on the sync queue: a DMA trigger costs
                # ~0.7us of engine time, which must not sit on the scalar
                # (exp) or gpsimd (mask) queues right when the loop starts
                lds.append(nc.sync.dma_start(out=qt[:], in_=qt_d[h]))
                lds.append(nc.sync.dma_start(out=kt[:], in_=kt_d[h]))
            vno = ldpool.tile([BLK, NB * VW], mmdt, tag="vn", name=f"vn{h}")
            vno3l = vno[:].rearrange("p (n c) -> p n c", c=VW)
            if h == 0:
                # first two key blocks first (PV j=0 needs them ~2us in)
                lds.append(nc.gpsimd.dma_start(
                    out=vno3l[:, 0:2, :], in_=v_d[h][:, 0:2, :]
                ))
                lds.append(nc.gpsimd.dma_start(
                    out=vno3l[:, 2:NB, :], in_=v_d[h][:, 2:NB, :]
                ))
            else:
                lds.append(nc.gpsimd.dma_start(out=vno3l[:], in_=v_d[h]))
            qts.append(qt)
            kts.append(kt)
            vnos.append(vno)
            load_insts.append(lds)
            if h == 0:
                # mask keep-tiles right after the head-0 loads on the sync
                # queue (before the gated later-head triggers, which block it)
                bts = []
                for u in range(n_bt):
                    t = const.tile([BLK, BLK], mmdt, tag=f"bt{u}", name=f"bt_sb{u}")
                    nc.sync.dma_start(out=t[:], in_=bt_d[u])
                    bts.append(t)

        qk_h = []
        pv_h = []
        epi_h = []
        epi0_h = []
        for h in range(HPC):
            qk_groups = []
            pv_groups = []
            qt, kt, vno = qts[h], kts[h], vnos[h]
            vno3 = vno[:].rearrange("p (n c) -> p n c", c=VW)

            # ---- main loop over key blocks ----
            # one PSUM accumulator tile per bank so each bank's drain only
            # depends on its own accumulation group
            pvh = [
                pvpool.tile([VW, 512], f32, tag="pv", name=f"pv{h}_{i}")
                for i in range(NBANK)
            ]
            for j in range(NB):
                blocks = active[j]
                if not blocks:
                    continue
                pT = ppool.tile([BLK, S], mmdt, tag="pT")
                qk_g = []
                for (r0, r1) in _runs(blocks):
                    for (c0, c1) in _ceil_pieces(r0 * BLK, r1 * BLK, 512):
                        w = c1 - c0
                        sc = scpool.tile([BLK, w], f32, tag="sc")
                        qk_g.append(nc.tensor.matmul(
                            sc[:, 0:w],
                            lhsT=kt[:, j * BLK:(j + 1) * BLK],
                            rhs=qt[:, c0:c1],
                            start=True,
                            stop=True,
                        ))
                        nc.scalar.activation(pT[:, c0:c1], sc[:, 0:w], Exp, scale=SCALE)
                qk_groups.append(qk_g)
                pv_g = []
                # mixed blocks: zero the masked probabilities (gpsimd is idle)
                for i in blocks:
                    if codes[i, j] == BIAS:
                        sl = pT[:, i * BLK:(i + 1) * BLK]
                        nc.gpsimd.tensor_tensor(sl, sl, bts[tile_idx[i, j]][:], mult)
                # PV accumulation: start/stop flags at PSUM-bank granularity
                bank_order = sorted(
                    range(NBANK),
                    key=lambda b: any(
                        codes[i, j] == BIAS
                        for i in range(b * 4, b * 4 + 4)
                        if i in blocks
                    ),
                )
                for bank in bank_order:
                    bi = [i for i in blocks if bank * 4 <= i < bank * 4 + 4]
                    if not bi:
                        continue
                    toff = bank * 512  # tile-relative offset of this bank
                    is_last = j == bank_last[bank]
                    if j == bank_first[bank]:
                        # first write: one full-bank matmul so every column
                        # starts with start=True; zero any inactive columns
                        # of pT first (no-op for causal/empty masks).
                        for i in range(bank * 4, bank * 4 + 4):
                            if i not in bi:
                                nc.gpsimd.memset(
                                    pT[:, i * BLK:(i + 1) * BLK], 0.0
                                )
                        g0, g1 = bank * 4 * BLK, (bank + 1) * 4 * BLK
                        pv_g.append(nc.tensor.matmul(
                            pvh[bank][:, g0 - toff:g1 - toff],
                            lhsT=vno3[:, j, :],
                            rhs=pT[:, g0:g1],
                            start=True,
                            stop=is_last,
                        ))
                    else:
                        runs = _runs(bi)
                        for ri, (r0, r1) in enumerate(runs):
                            pv_g.append(nc.tensor.matmul(
                                pvh[bank][:, r0 * BLK - toff:r1 * BLK - toff],
                                lhsT=vno3[:, j, :],
                                rhs=pT[:, r0 * BLK:r1 * BLK],
                                start=False,
                                stop=is_last and ri == len(runs) - 1,
                            ))
                pv_groups.append(pv_g)

            # ---- epilogue: normalize and write out, per PSUM bank ----
            # Bank b (q blocks 4b..4b+3) finishes accumulating at its last
            # key block (j=3/7/11/15 for causal), so its drain -- PSUM->SBUF
            # copy, retranspose, denominator reciprocal, normalize, DMA out
            # -- is scheduled right behind that point in the k-loop instead
            # of all piling up at the end of the head. All staging is fp16:
            # the PE transposes run at 1 cyc/col and SBUF traffic halves.
            ot = otpool.tile([VW, S], mmdt, tag="ot")
            epi_banks = []
            for bank in range(NBANK):
                lst = []
                nc.vector.tensor_copy(
                    ot[:, bank * 512:(bank + 1) * 512],
                    pvh[bank][:, :],
                )
                rt = rtpool.tile([BLK, 264], mmdt, tag="rt", name=f"rt{h}_{bank}")
                for u in range(4):
                    i = bank * 4 + u
                    lst.append(nc.tensor.transpose(
                        rt[:, u * D:(u + 1) * D],
                        ot[0:D, i * BLK:(i + 1) * BLK],
                        identm[0:D, 0:D],
                    ))
                # gather denominators [1, 512] -> [4, 128] -> [128, 4]
                dq = smpool.tile([4, BLK], mmdt, tag="dq")
                nc.sync.dma_start(
                    out=dq[:], in_=ot[D:VW, bank * 512:(bank + 1) * 512]
                )
                lst.append(nc.tensor.transpose(
                    rt[:, 256:260], dq[:], identm[0:4, 0:4]
                ))
                rcp = smpool.tile([BLK, 4], f32, tag="rcp")
                nc.vector.reciprocal(rcp[:], rt[:, 256:260])
                osb = outpool.tile([BLK, 256], mmdt, tag="ob")
                nc.vector.tensor_tensor(
                    osb[:].rearrange("p (u d) -> p u d", d=D),
                    rt[:, 0:256].rearrange("p (u d) -> p u d", d=D),
                    rcp[:]
                    .rearrange("p (u o) -> p u o", o=1)
                    .broadcast_to([BLK, 4, D]),
                    mult,
                )
                nc.sync.dma_start(
                    out=o_d[h].rearrange("(n p) d -> p n d", p=BLK)[
                        :, bank * 4:(bank + 1) * 4, :
                    ],
                    in_=osb[:].rearrange("p (u d) -> p u d", d=D),
                )
                epi_banks.append(lst)
            qk_h.append(qk_groups)
            pv_h.append(pv_groups)
            epi_h.append(epi_banks)

        # Later heads' input loads wait for head 0's first matmul: the DMA
        # engines round-robin active transfers, so an ungated 1.8MB preload
        # burst would delay the first K/Q tiles (and the first matmul) by
        # several microseconds.
        first_mm = qk_h[0][0][0]
        for h in range(1, HPC):
            for ld in load_insts[h]:
                add_dep_helper(ld.ins, first_mm.ins, sync=True, reason="preload gate")

        # Build the PE ordering chain. Bank b's drain (retranspose +
        # denominator transpose) slots in two key-blocks after the bank's
        # last PV write; the final bank's drain slots into the NEXT head's
        # stream (its PSUM->SBUF copy needs ~0.7us of DVE time first).
        chain = []
        for h in range(HPC):
            qk = qk_h[h]
            pv = pv_h[h]
            assert len(qk) == len(pv)
            pending = {b: bank_last[b] + 2 for b in range(NBANK)}
            if qk:
                chain += qk[0]
            for idx in range(1, len(qk)):
                chain += qk[idx]
                for b, at in list(pending.items()):
                    if idx == at:
                        chain += epi_h[h][b]
                        del pending[b]
                chain += pv[idx - 1]
            if pv:
                chain += pv[-1]
            for b in sorted(pending):
                chain += epi_h[h][b]
        for a, b in zip(chain, chain[1:]):
            add_dep_helper(b.ins, a.ins, sync=False, reason="pe weight-group order")
    nc.compile()
    return nc


MM_DT = __import__("os").environ.get("ATTN_MM_DT", "float16")


def _get_program(mask):
    codes, tile_idx, bt = _plan_from_mask(mask)
    key = (codes.tobytes(), tile_idx.tobytes(), bt.tobytes(), MM_DT)
    if key not in _cache:
        _cache[key] = (build_nc(codes, tile_idx, bt.shape[0], MM_DT), bt)
    return _cache[key]


LAST_RESULTS = None  # BassKernelResults of the most recent run (for profiling)


def kernel(q, k, v, mask):
    global LAST_RESULTS
    from concourse.bass_utils import run_bass_kernel_spmd
    import ml_dtypes

    npdt = {"float16": np.float16, "bfloat16": ml_dtypes.bfloat16}[MM_DT]
    nc, bt = _get_program(mask)
    qf = np.asarray(q, np.float32).reshape(BH, S, D)
    kf = np.asarray(k, np.float32).reshape(BH, S, D)
    vf = np.asarray(v, np.float32).reshape(BH, S, D)
    # V in its SBUF layout: [128, nb, 65] per head, ones column baked in.
    vr = vf.reshape(BH, NB, BLK, D).transpose(0, 2, 1, 3)  # [BH, 128, NB, D]
    vno = np.concatenate(
        [vr, np.ones((BH, BLK, NB, 1), np.float32)], axis=3
    ).astype(npdt)  # [BH, BLK, NB, VW] block-major
    bt16 = bt.astype(npdt)
    in_maps = []
    for c in range(NCORES):
        sl = slice(c * HPC, (c + 1) * HPC)
        in_maps.append({
            # per-shard layout: Q/K shipped [head, d, seq], pre-cast
            "qt": np.ascontiguousarray(qf[sl].transpose(0, 2, 1)).astype(npdt),
            "kt": np.ascontiguousarray(kf[sl].transpose(0, 2, 1)).astype(npdt),
            "v": vno[sl],
            "bt": bt16,
        })
    res = run_bass_kernel_spmd(nc, in_maps, list(range(NCORES)))
    LAST_RESULTS = res
    out = np.concatenate(
        [np.asarray(res.results[c]["o"]) for c in range(NCORES)], axis=0
    )
    return out.reshape(B, H, S, D).astype(np.float32)


# revision 25
# speedup vs baseline: 1.1715x; 1.1715x over previous
# Multi-head causal attention for 8 Trainium2 NeuronCores (Bass/Tile).
#
# Problem: q,k,v [2,16,2048,64] f32, bool mask [1,1,2048,2048] (True = masked,
# additive -1e4 bias before softmax in the reference).
#
# Sharding: batch*heads = 32 items, 4 per core (pure data/head parallel, no
# communication).
#
# Per-core kernel (per head), all in "transposed score" layout so softmax'd
# probabilities come out of the ScalarEngine already laid out for the PV
# matmul (keys on partitions):
#   - Q,K arrive pre-transposed from the host ([head, d, seq] fp16), V arrives
#     pre-arranged in its exact SBUF layout [128, nb, 65] with a ones column
#     baked in (so every input DMA is one large contiguous transfer -- the
#     strided V gather used to serialize ~13us of DMA at kernel start).
#   - Per key-block j: S^T_j = K_j Q^T via matmul into PSUM [128, <=1024];
#     exp on ScalarE with the 1/sqrt(64) scale folded in (no row-max
#     subtraction: |scores| <= ~7, exp is safe in f32, and softmax is
#     shift-invariant so the result matches the reference).
#   - Mask handling, decided on the host per 128x128 block from the actual
#     mask input: fully-masked blocks are skipped outright (their probs
#     underflow to exactly 0 in the reference too); mixed blocks multiply
#     the probabilities by a 0/1 keep-tile (equivalent to the -1e4 bias:
#     exp(s - 1e4) == 0 exactly in f32) on the otherwise idle GpSimd engine.
#   - PV accumulates O^T [65, q] in PSUM over key-blocks, with V augmented
#     by a ones-column so row 64 of the accumulator is the softmax
#     denominator for free.
#   - Epilogue (all fp16 staging -- keeps the PE transposes at 1 cyc/col):
#     GpSimd copies the PSUM accumulator halves to SBUF, TensorE transposes
#     O^T back, DVE multiplies by the reciprocal denominator (gathered to
#     [128,16] via a tiny SBUF->SBUF DMA), DMA out.
#   - The PE instruction stream is chained (sync=False deps) in a software-
#     pipelined order so the TensorEngine -- the bottleneck at its throttled
#     sustained rate of ~1 col / 0.83ns -- never waits: QK_j+1 before PV_j,
#     epilogue transposes of head h slotted into head h+1's QK stream.
import numpy as np
from contextlib import ExitStack

B, H, S, D = 2, 16, 2048, 64
NCORES = 8
BH = B * H
HPC = BH // NCORES  # heads per core
BLK = 128
NB = S // BLK  # 16
VW = D + 1  # V columns + ones column
SCALE = 1.0 / 8.0  # 1/sqrt(D)

FREE, SKIP, BIAS = 0, 1, 2

_cache = {}


def _plan_from_mask(mask):
    """Classify 128x128 mask blocks; build unique 0/1 keep-tiles ([key, query]
    orientation) for the mixed blocks."""
    mask2d = np.asarray(mask).reshape(S, S).astype(bool)
    m = mask2d.reshape(NB, BLK, NB, BLK)
    anyb = m.any(axis=(1, 3))
    allb = m.all(axis=(1, 3))
    codes = np.where(allb, SKIP, np.where(anyb, BIAS, FREE)).astype(np.int64)
    # A query row whose whole key range is masked sees a constant bias, which
    # softmax ignores -- the reference then equals unmasked attention. Treat
    # whole such q-blocks as unmasked.
    fq = mask2d.all(axis=1).reshape(NB, BLK).all(axis=1)
    codes[fq, :] = FREE
    tiles = {}
    tile_idx = np.full((NB, NB), -1, dtype=np.int64)
    for qb in range(NB):
        for kb in range(NB):
            if codes[qb, kb] != BIAS:
                continue
            t = np.ascontiguousarray(
                (~mask2d[qb * BLK:(qb + 1) * BLK, kb * BLK:(kb + 1) * BLK].T)
            ).astype(np.float32)
            key = t.tobytes()
            if key not in tiles:
                tiles[key] = (len(tiles), t)
            tile_idx[qb, kb] = tiles[key][0]
    if tiles:
        bt = np.stack([t for _, t in sorted(tiles.values())], axis=0)
    else:
        bt = np.zeros((1, BLK, BLK), np.float32)
    return codes, tile_idx, bt


def _ceil_pieces(c0, c1, step):
    out = []
    c = c0
    while c < c1:
        out.append((c, min(c + step, c1)))
        c = out[-1][1]
    return out


def _runs(blocks):
    """Contiguous runs from a sorted list of block indices."""
    runs = []
    for i in blocks:
        if runs and runs[-1][1] == i:
            runs[-1][1] = i + 1
        else:
            runs.append([i, i + 1])
    return [tuple(r) for r in runs]


def build_nc(codes, tile_idx, n_bt, mmdt_name="float16"):
    import concourse.bass as bass
    import concourse.mybir as mybir
    import concourse.tile as tile
    from concourse import bacc
    from concourse.masks import make_identity
    from concourse.tile_rust import add_dep_helper

    dt = mybir.dt
    f32 = dt.float32
    mmdt = getattr(dt, mmdt_name)
    Exp = mybir.ActivationFunctionType.Exp
    mult = mybir.AluOpType.mult

    # Per key-block: which q-blocks participate.
    active = {j: [i for i in range(NB) if codes[i, j] != SKIP] for j in range(NB)}
    for i in range(NB):
        assert any(codes[i, j] != SKIP for j in range(NB)), (
            "query block with all key blocks masked should be impossible"
        )
    # PV PSUM accumulation start/stop must be managed per 512-column PSUM
    # bank (4 q-blocks): first/last key-block writing each bank.
    NBANK = 4
    bank_first = {}
    bank_last = {}
    for bank in range(NBANK):
        js = [
            j
            for j in range(NB)
            if any(codes[i, j] != SKIP for i in range(bank * 4, bank * 4 + 4))
        ]
        bank_first[bank] = js[0]
        bank_last[bank] = js[-1]

    nc = bacc.Bacc("TRN2", target_bir_lowering=False, debug=False, num_devices=NCORES)
    # Q and K arrive pre-transposed ([head, d, seq]) and V pre-arranged in
    # its SBUF layout ([128, nb*65] with the ones column baked in), all
    # pre-cast to the matmul dtype -- every load is one contiguous DMA.
    qt_d = nc.dram_tensor("qt", [HPC, D, S], mmdt, kind="ExternalInput").ap()
    kt_d = nc.dram_tensor("kt", [HPC, D, S], mmdt, kind="ExternalInput").ap()
    v_d = nc.dram_tensor("v", [HPC, BLK, NB, VW], mmdt, kind="ExternalInput").ap()
    bt_d = nc.dram_tensor("bt", [n_bt, BLK, BLK], mmdt, kind="ExternalInput").ap()
    o_d = nc.dram_tensor("o", [HPC, S, D], mmdt, kind="ExternalOutput").ap()

    with tile.TileContext(nc) as tc, ExitStack() as ctx:
        const = ctx.enter_context(tc.tile_pool(name="const", bufs=1))
        ldpool = ctx.enter_context(tc.tile_pool(name="ld", bufs=HPC))
        tpool = ctx.enter_context(tc.tile_pool(name="tp", bufs=HPC))
        ppool = ctx.enter_context(tc.tile_pool(name="pp", bufs=3))
        otpool = ctx.enter_context(tc.tile_pool(name="ot", bufs=2))
        smpool = ctx.enter_context(tc.tile_pool(name="sm", bufs=3))
        outpool = ctx.enter_context(tc.tile_pool(name="ob", bufs=4))
        # PSUM budget (8 banks x 2KB/partition): sc 3 banks + pv 4 + rt 1
        # (the denominator transpose shares the rt bank at cols 512:520).
        # rt/dnt in their own rings (not the pv ring) so the next head's PV
        # accumulation never serializes behind this head's epilogue drain.
        scpool = ctx.enter_context(tc.tile_pool(name="sc", bufs=3, space="PSUM"))
        pvpool = ctx.enter_context(tc.tile_pool(name="pv", bufs=4, space="PSUM"))
        rtpool = ctx.enter_context(tc.tile_pool(name="rt", bufs=1, space="PSUM"))

        identm = const.tile([BLK, BLK], mmdt, tag="identm")
        make_identity(nc, identm[:])

        # PE-order bookkeeping: weight reloads cost ~330ns per stationary
        # swap, so same-weight matmuls must run adjacently. We collect the
        # PE instructions and chain them (sync=False deps) in a software-
        # pipelined order: QK_j+1 before PV_j, epilogue transposes of head h
        # slotted early into head h+1's stream.
        # ---- input loads for all heads, hoisted ahead of the compute so no
        # load DMA trigger ever sits behind a blocking compute instruction on
        # its engine queue. The DMA hardware round-robins active transfers,
        # so a transfer's latency scales with how many are in flight: head
        # 0's loads are split fine (k block 0 and the first q piece finish
        # fast, letting the PE start ~4us earlier) and later heads' loads are
        # gated on head 0's first matmul (via add_dep below) so they don't
        # steal preamble bandwidth.
        qts, kts, vnos, load_insts = [], [], [], []
        gated_lds = []
        for h in range(HPC):
            lds = []
            qt = tpool.tile([D, S], mmdt, tag="qt", name=f"qt{h}")
            kt = tpool.tile([D, S], mmdt, tag="kt", name=f"kt{h}")
            if h == 0:
                # head 0 split fine: k block 0 and small q pieces finish
                # fast so the first QK matmul fires ASAP; kt on the scalar
                # queue (idle in the preamble), qt on sync
                lds.append(nc.scalar.dma_start(out=kt[:, 0:BLK], in_=kt_d[h][:, 0:BLK]))
                for (c0, c1) in [(0, 256), (256, 512), (512, 1024), (1024, 1536), (1536, 2048)]:
                    lds.append(nc.sync.dma_start(
                        out=qt[:, c0:c1], in_=qt_d[h][:, c0:c1]
                    ))
                lds.append(nc.scalar.dma_start(out=kt[:, BLK:], in_=kt_d[h][:, BLK:]))
            else:
                # later heads entirely on the sync queue: a DMA trigger costs
                # ~0.7us of engine time, which must not sit on the scalar
                # (exp) or gpsimd (mask) queues right when the loop starts
                lds.append(nc.sync.dma_start(out=qt[:], in_=qt_d[h]))
                lds.append(nc.sync.dma_start(out=kt[:], in_=kt_d[h]))
            vno = ldpool.tile([BLK, NB * VW], mmdt, tag="vn", name=f"vn{h}")
            vno3l = vno[:].rearrange("p (n c) -> p n c", c=VW)
            if h == 0:
                # first two key blocks first (PV j=0 needs them ~2us in);
                # the bulk is gated with the later heads' loads (not needed
                # until PV j=2, well after the first matmul)
                lds.append(nc.gpsimd.dma_start(
                    out=vno3l[:, 0:2, :], in_=v_d[h][:, 0:2, :]
                ))
                gated_lds.append(nc.gpsimd.dma_start(
                    out=vno3l[:, 2:NB, :], in_=v_d[h][:, 2:NB, :]
                ))
            else:
                lds.append(nc.gpsimd.dma_start(out=vno3l[:], in_=v_d[h]))
            qts.append(qt)
            kts.append(kt)
            vnos.append(vno)
            load_insts.append(lds)
            if h == 0:
                # mask keep-tiles right after the head-0 loads on the sync
                # queue (before the gated later-head triggers, which block it)
                bts = []
                for u in range(n_bt):
                    t = const.tile([BLK, BLK], mmdt, tag=f"bt{u}", name=f"bt_sb{u}")
                    nc.sync.dma_start(out=t[:], in_=bt_d[u])
                    bts.append(t)

        qk_h = []
        pv_h = []
        epi_h = []
        epi0_h = []
        for h in range(HPC):
            qk_groups = []
            pv_groups = []
            qt, kt, vno = qts[h], kts[h], vnos[h]
            vno3 = vno[:].rearrange("p (n c) -> p n c", c=VW)

            # ---- main loop over key blocks ----
            # one PSUM accumulator tile per bank so each bank's drain only
            # depends on its own accumulation group
            pvh = [
                pvpool.tile([VW, 512], f32, tag="pv", name=f"pv{h}_{i}")
                for i in range(NBANK)
            ]
            for j in range(NB):
                blocks = active[j]
                if not blocks:
                    continue
                pT = ppool.tile([BLK, S], mmdt, tag="pT")
                qk_g = []
                for (r0, r1) in _runs(blocks):
                    for (c0, c1) in _ceil_pieces(r0 * BLK, r1 * BLK, 512):
                        w = c1 - c0
                        sc = scpool.tile([BLK, w], f32, tag="sc")
                        qk_g.append(nc.tensor.matmul(
                            sc[:, 0:w],
                            lhsT=kt[:, j * BLK:(j + 1) * BLK],
                            rhs=qt[:, c0:c1],
                            start=True,
                            stop=True,
                        ))
                        nc.scalar.activation(pT[:, c0:c1], sc[:, 0:w], Exp, scale=SCALE)
                qk_groups.append(qk_g)
                pv_g = []
                # mixed blocks: zero the masked probabilities (gpsimd is idle)
                for i in blocks:
                    if codes[i, j] == BIAS:
                        sl = pT[:, i * BLK:(i + 1) * BLK]
                        nc.gpsimd.tensor_tensor(sl, sl, bts[tile_idx[i, j]][:], mult)
                # PV accumulation: start/stop flags at PSUM-bank granularity
                bank_order = sorted(
                    range(NBANK),
                    key=lambda b: any(
                        codes[i, j] == BIAS
                        for i in range(b * 4, b * 4 + 4)
                        if i in blocks
                    ),
                )
                for bank in bank_order:
                    bi = [i for i in blocks if bank * 4 <= i < bank * 4 + 4]
                    if not bi:
                        continue
                    toff = bank * 512  # tile-relative offset of this bank
                    is_last = j == bank_last[bank]
                    if j == bank_first[bank]:
                        # first write: one full-bank matmul so every column
                        # starts with start=True; zero any inactive columns
                        # of pT first (no-op for causal/empty masks).
                        for i in range(bank * 4, bank * 4 + 4):
                            if i not in bi:
                                nc.gpsimd.memset(
                                    pT[:, i * BLK:(i + 1) * BLK], 0.0
                                )
                        g0, g1 = bank * 4 * BLK, (bank + 1) * 4 * BLK
                        pv_g.append(nc.tensor.matmul(
                            pvh[bank][:, g0 - toff:g1 - toff],
                            lhsT=vno3[:, j, :],
                            rhs=pT[:, g0:g1],
                            start=True,
                            stop=is_last,
                        ))
                    else:
                        runs = _runs(bi)
                        for ri, (r0, r1) in enumerate(runs):
                            pv_g.append(nc.tensor.matmul(
                                pvh[bank][:, r0 * BLK - toff:r1 * BLK - toff],
                                lhsT=vno3[:, j, :],
                                rhs=pT[:, r0 * BLK:r1 * BLK],
                                start=False,
                                stop=is_last and ri == len(runs) - 1,
                            ))
                pv_groups.append(pv_g)

            # ---- epilogue: normalize and write out, per PSUM bank ----
            # Bank b (q blocks 4b..4b+3) finishes accumulating at its last
            # key block (j=3/7/11/15 for causal), so its drain -- PSUM->SBUF
            # copy, retranspose, denominator reciprocal, normalize, DMA out
            # -- is scheduled right behind that point in the k-loop instead
            # of all piling up at the end of the head. All staging is fp16:
            # the PE transposes run at 1 cyc/col and SBUF traffic halves.
            ot = otpool.tile([VW, S], mmdt, tag="ot")
            epi_banks = []
            for bank in range(NBANK):
                lst = []
                nc.vector.tensor_copy(
                    ot[:, bank * 512:(bank + 1) * 512],
                    pvh[bank][:, :],
                )
                rt = rtpool.tile([BLK, 264], mmdt, tag="rt", name=f"rt{h}_{bank}")
                for u in range(4):
                    i = bank * 4 + u
                    lst.append(nc.tensor.transpose(
                        rt[:, u * D:(u + 1) * D],
                        ot[0:D, i * BLK:(i + 1) * BLK],
                        identm[0:D, 0:D],
                    ))
                # gather denominators [1, 512] -> [4, 128] -> [128, 4]
                dq = smpool.tile([4, BLK], mmdt, tag="dq")
                nc.sync.dma_start(
                    out=dq[:], in_=ot[D:VW, bank * 512:(bank + 1) * 512]
                )
                lst.append(nc.tensor.transpose(
                    rt[:, 256:260], dq[:], identm[0:4, 0:4]
                ))
                rcp = smpool.tile([BLK, 4], f32, tag="rcp")
                nc.vector.reciprocal(rcp[:], rt[:, 256:260])
                osb = outpool.tile([BLK, 256], mmdt, tag="ob")
                nc.vector.tensor_tensor(
                    osb[:].rearrange("p (u d) -> p u d", d=D),
                    rt[:, 0:256].rearrange("p (u d) -> p u d", d=D),
                    rcp[:]
                    .rearrange("p (u o) -> p u o", o=1)
                    .broadcast_to([BLK, 4, D]),
                    mult,
                )
                nc.sync.dma_start(
                    out=o_d[h].rearrange("(n p) d -> p n d", p=BLK)[
                        :, bank * 4:(bank + 1) * 4, :
                    ],
                    in_=osb[:].rearrange("p (u d) -> p u d", d=D),
                )
                epi_banks.append(lst)
            qk_h.append(qk_groups)
            pv_h.append(pv_groups)
            epi_h.append(epi_banks)

        # Later heads' input loads wait for head 0's first matmul: the DMA
        # engines round-robin active transfers, so an ungated 1.8MB preload
        # burst would delay the first K/Q tiles (and the first matmul) by
        # several microseconds.
        first_mm = qk_h[0][0][0]
        for ld in gated_lds + [ld for h in range(1, HPC) for ld in load_insts[h]]:
            add_dep_helper(ld.ins, first_mm.ins, sync=True, reason="preload gate")

        # Build the PE ordering chain. Bank b's drain (retranspose +
        # denominator transpose) slots in two key-blocks after the bank's
        # last PV write; the final bank's drain slots into the NEXT head's
        # stream (its PSUM->SBUF copy needs ~0.7us of DVE time first).
        chain = []
        for h in range(HPC):
            qk = qk_h[h]
            pv = pv_h[h]
            assert len(qk) == len(pv)
            pending = {b: bank_last[b] + 3 for b in range(NBANK - 1)}
            if qk:
                chain += qk[0]
            for idx in range(1, len(qk)):
                chain += qk[idx]
                if idx == 3 and h > 0:
                    chain += epi_h[h - 1][NBANK - 1]
                for b, at in list(pending.items()):
                    if idx == at:
                        chain += epi_h[h][b]
                        del pending[b]
                chain += pv[idx - 1]
            if pv:
                chain += pv[-1]
            for b in sorted(pending):
                chain += epi_h[h][b]
        chain += epi_h[HPC - 1][NBANK - 1]
        for a, b in zip(chain, chain[1:]):
            add_dep_helper(b.ins, a.ins, sync=False, reason="pe weight-group order")
    nc.compile()
    return nc


MM_DT = __import__("os").environ.get("ATTN_MM_DT", "float16")


def _get_program(mask):
    codes, tile_idx, bt = _plan_from_mask(mask)
    key = (codes.tobytes(), tile_idx.tobytes(), bt.tobytes(), MM_DT)
    if key not in _cache:
        _cache[key] = (build_nc(codes, tile_idx, bt.shape[0], MM_DT), bt)
    return _cache[key]


LAST_RESULTS = None  # BassKernelResults of the most recent run (for profiling)


def kernel(q, k, v, mask):
    global LAST_RESULTS
    from concourse.bass_utils import run_bass_kernel_spmd
    import ml_dtypes

    npdt = {"float16": np.float16, "bfloat16": ml_dtypes.bfloat16}[MM_DT]
    nc, bt = _get_program(mask)
    qf = np.asarray(q, np.float32).reshape(BH, S, D)
    kf = np.asarray(k, np.float32).reshape(BH, S, D)
    vf = np.asarray(v, np.float32).reshape(BH, S, D)
    # V in its SBUF layout: [128, nb, 65] per head, ones column baked in.
    vr = vf.reshape(BH, NB, BLK, D).transpose(0, 2, 1, 3)  # [BH, 128, NB, D]
    vno = np.concatenate(
        [vr, np.ones((BH, BLK, NB, 1), np.float32)], axis=3
    ).astype(npdt)  # [BH, BLK, NB, VW] block-major
    bt16 = bt.astype(npdt)
    in_maps = []
    for c in range(NCORES):
        sl = slice(c * HPC, (c + 1) * HPC)
        in_maps.append({
            # per-shard layout: Q/K shipped [head, d, seq], pre-cast
            "qt": np.ascontiguousarray(qf[sl].transpose(0, 2, 1)).astype(npdt),
            "kt": np.ascontiguousarray(kf[sl].transpose(0, 2, 1)).astype(npdt),
            "v": vno[sl],
            "bt": bt16,
        })
    res = run_bass_kernel_spmd(nc, in_maps, list(range(NCORES)))
    LAST_RESULTS = res
    out = np.concatenate(
        [np.asarray(res.results[c]["o"]) for c in range(NCORES)], axis=0
    )
    return out.reshape(B, H, S, D).astype(np.float32)
